# revision 27
# baseline (speedup 1.0000x reference)
"""GPT block (LN -> causal MHA -> LN -> MLP) on 8 TRN2 NeuronCores.

Sharding: each core owns one (batch, zig-zag query-chunk-pair); B=4 x 2
pairs = 8 cores. K/V are recomputed per core for the full sequence (no
collectives); per-core causality uses a block permutation of the
sequence with static diagonal masks and per-core exp scale/bias inputs.

Engine/pipeline structure:
  - LN: bf16 stats via ones-column matmuls, scalar-engine row chains,
    reciprocal_approx_fast, broadcast mu/rstd once per chunk, 3-op DVE
    apply. LN1 chunk 0 first, remaining chunks interleaved with QKV.
  - GEMM dtypes: QKV runs fp8e4 DoubleRow with x32-scaled weights
    (4 matmuls per 1024-contract instead of 8; the 1/32 is folded into
    the bias/copy ops). FC / proj / attnproj stay bf16 -- on this part
    a matmul instruction costs the same regardless of dtype, so fp8
    only pays where it halves the instruction count. K bias dropped
    (softmax-invariant), 1/sqrt(d) folded into Q.
  - Attention: q/k stored fp8e4; 2-head row-packed score matmuls, exp
    batched over kv-tile pairs ([128,1024] ACT), multiplicative bf16
    diagonal masks after exp, softmax denominator via ones-column in
    the AV matmul + broadcast + reciprocal_approx_fast; the per-head
    finalize is deferred into the next head's score phase.
  - Interleave for HAM warmth: QKV for perm positions {0,2} first,
    attention slot A with remaining QKV as PE filler, attention slot B
    with attnproj-A as filler, then attnproj-B || LN2, FC, and a
    two-half-pass proj tail.
"""

import numpy as np
import ml_dtypes

BF = ml_dtypes.bfloat16

E = 1024          # embedding
T = 2048          # sequence
B = 4             # batch
NH = 16           # heads
D = 64            # head dim
HID = 4096        # mlp hidden
KT = E // 128     # k-tiles over embedding (8)
CH = 512          # chunk rows
NEG = -1.0e9
EPS = 1e-5

_CACHE = {}


def _build_program():
    import concourse.bass as bass
    import concourse.tile as tile
    from concourse import bacc, mybir

    f32 = mybir.dt.float32
    f32r = mybir.dt.float32r
    bf16 = mybir.dt.bfloat16
    fp8 = mybir.dt.float8e4
    DR = mybir.MatmulPerfMode.DoubleRow
    AF = mybir.ActivationFunctionType
    ALU = mybir.AluOpType

    nc = bacc.Bacc()

    xT_d = nc.declare_dram_parameter("xT", [E, T], f32, isOutput=False)
    w8a_d = nc.declare_dram_parameter("w8_attn", [E, 3 * E], fp8, isOutput=False)
    b_q_d = nc.declare_dram_parameter("b_q", [E, 1], f32, isOutput=False)
    b_v_d = nc.declare_dram_parameter("b_v", [1, E], bf16, isOutput=False)
    w_ap_d = nc.declare_dram_parameter("w_ap", [E, E], bf16, isOutput=False)
    b_ap_d = nc.declare_dram_parameter("b_ap", [E, 1], f32, isOutput=False)
    ln1_g_d = nc.declare_dram_parameter("ln1_g", [E, 1], f32, isOutput=False)
    ln1_b_d = nc.declare_dram_parameter("ln1_b", [E, 1], f32, isOutput=False)
    ln2_g_d = nc.declare_dram_parameter("ln2_g", [E, 1], f32, isOutput=False)
    ln2_b_d = nc.declare_dram_parameter("ln2_b", [E, 1], f32, isOutput=False)
    w_fc_d = nc.declare_dram_parameter("w_fc", [E, HID], bf16, isOutput=False)
    b_fc_d = nc.declare_dram_parameter("b_fc", [HID, 1], f32, isOutput=False)
    w_pr_d = nc.declare_dram_parameter("w_proj", [HID, E], bf16, isOutput=False)
    b_pr_d = nc.declare_dram_parameter("b_proj", [E, 1], f32, isOutput=False)
    dmask_d = nc.declare_dram_parameter("dmask01", [4, 128, CH], bf16, isOutput=False)
    sA_s_d = nc.declare_dram_parameter("sA_scale", [128, 1], f32, isOutput=False)
    sA_b_d = nc.declare_dram_parameter("sA_bias", [128, 1], f32, isOutput=False)
    sB_s_d = nc.declare_dram_parameter("sB_scale", [128, 3], f32, isOutput=False)
    sB_b_d = nc.declare_dram_parameter("sB_bias", [128, 3], f32, isOutput=False)
    out_d = nc.declare_dram_parameter("outT", [E, 2 * CH], f32, isOutput=True)

    w8a_r = w8a_d.rearrange("(k p) n -> p k n", p=128)
    w_ap_r = w_ap_d.rearrange("(k p) n -> p k n", p=128)
    w_fc_r = w_fc_d.rearrange("(k p) n -> p k n", p=128)

    with tile.TileContext(nc) as tc:
        from contextlib import ExitStack

        stack = ExitStack()
        with stack:
            const = stack.enter_context(tc.tile_pool(name="const", bufs=1))

            ones_col_f = const.tile([128, 1], f32)
            nc.vector.memset(ones_col_f[:], 1.0)
            ones_col_bf = const.tile([128, 1], bf16)
            nc.vector.memset(ones_col_bf[:], 1.0)
            ones_row_bf = const.tile([1, 64], bf16)
            nc.vector.memset(ones_row_bf[:], 1.0)
            ones_row_f = const.tile([1, 128], f32)
            nc.vector.memset(ones_row_f[:], 1.0)
            eps_t = const.tile([1, 1], f32)
            nc.vector.memset(eps_t[:], EPS)

            dmask = const.tile([128, 4, CH], bf16)
            nc.sync.dma_start(dmask[:], dmask_d.rearrange("v p n -> p v n"))
            sA_s = const.tile([128, 1], f32)
            nc.sync.dma_start(sA_s[:], sA_s_d[:])
            sA_b = const.tile([128, 1], f32)
            nc.sync.dma_start(sA_b[:], sA_b_d[:])
            sB_s = const.tile([128, 3], f32)
            nc.sync.dma_start(sB_s[:], sB_s_d[:])
            sB_b = const.tile([128, 3], f32)
            nc.sync.dma_start(sB_b[:], sB_b_d[:])

            ln1_gr = const.tile([128, KT, 1], f32)
            nc.sync.dma_start(ln1_gr[:], ln1_g_d.rearrange("(k p) o -> p k o", p=128))
            ln2_gr = const.tile([128, KT, 1], f32)
            nc.sync.dma_start(ln2_gr[:], ln2_g_d.rearrange("(k p) o -> p k o", p=128))
            ones_row_bf128 = const.tile([1, 128], bf16)
            nc.vector.memset(ones_row_bf128[:], 1.0)
            ln1_b = const.tile([128, KT, 1], f32)
            nc.sync.dma_start(ln1_b[:], ln1_b_d.rearrange("(k p) o -> p k o", p=128))
            ln2_b = const.tile([128, KT, 1], f32)
            nc.sync.dma_start(ln2_b[:], ln2_b_d.rearrange("(k p) o -> p k o", p=128))
            b_q = const.tile([128, 8, 1], f32)
            nc.sync.dma_start(b_q[:], b_q_d.rearrange("(k p) o -> p k o", p=128))
            b_v = const.tile([1, E], bf16)
            nc.sync.dma_start(b_v[:], b_v_d[:])
            b_ap = const.tile([128, KT, 1], f32)
            nc.sync.dma_start(b_ap[:], b_ap_d.rearrange("(k p) o -> p k o", p=128))
            b_fc = const.tile([128, 32, 1], f32)
            nc.sync.dma_start(b_fc[:], b_fc_d.rearrange("(k p) o -> p k o", p=128))
            b_pr = const.tile([128, KT, 1], f32)
            nc.sync.dma_start(b_pr[:], b_pr_d.rearrange("(k p) o -> p k o", p=128))

            # ---- long-lived pools (open order = reverse close order) ----
            gsb = stack.enter_context(tc.tile_pool(name="gsb", bufs=2))
            x2p = stack.enter_context(tc.tile_pool(name="x2p", bufs=1))
            x2T = x2p.tile([128, KT, 2 * CH], bf16)
            h2p = stack.enter_context(tc.tile_pool(name="h2p", bufs=1))
            h2B = h2p.tile([128, KT, 2 * CH], bf16)
            app = stack.enter_context(tc.tile_pool(name="app", bufs=2))
            attn_ctx = ExitStack()
            attnp = attn_ctx.enter_context(tc.tile_pool(name="attnp", bufs=1))
            attnT = attnp.tile([128, KT, 2 * CH], bf16)

            # PSUM: gemm pool at stack bottom (lives past attention)
            gemm_ps_ctx = ExitStack()
            gps = gemm_ps_ctx.enter_context(
                tc.tile_pool(name="gps", bufs=2, space="PSUM"))

            qkv_ctx = ExitStack()
            qkvp = qkv_ctx.enter_context(tc.tile_pool(name="qkvp", bufs=1))
            qT = qkvp.tile([128, KT, 2 * CH], fp8)
            kT = qkvp.tile([128, KT, T], fp8)
            v_aug = qkvp.tile([128, 16, NH * 65], bf16)
            v4 = v_aug.rearrange("p m (h w) -> p m h w", h=NH)

            atsb_ctx = ExitStack()
            atsb = atsb_ctx.enter_context(tc.tile_pool(name="atsb", bufs=4))

            # pools that die at end of attention slot A
            p2_ctx = ExitStack()
            ln1o_p = p2_ctx.enter_context(tc.tile_pool(name="ln1o", bufs=1))
            ln1_odd = ln1o_p.tile([128, KT, 2, CH], fp8)   # perm pos {1,3}
            wqk = p2_ctx.enter_context(tc.tile_pool(name="wqk", bufs=2))
            qksb = p2_ctx.enter_context(tc.tile_pool(name="qksb", bufs=2))

            p2a_ctx = ExitStack()
            ln1e_p = p2a_ctx.enter_context(tc.tile_pool(name="ln1e", bufs=1))
            ln1_even = ln1e_p.tile([128, KT, 2, CH], fp8)  # perm pos {0,2}

            def ln1_slice(kt, ch):
                t = ln1_even if ch in (0, 2) else ln1_odd
                return t[:, kt, ch // 2 if ch in (0, 2) else (ch - 1) // 2, :]

            def ln1_dr(kp, ch):
                t = ln1_even if ch in (0, 2) else ln1_odd
                return t[:, 2 * kp:2 * kp + 2,
                         ch // 2 if ch in (0, 2) else (ch - 1) // 2, :]

            def ln_units(dst_fn, src_fn, ch, g_row, b_col, src_bf,
                         lnps, lnsb, skipg=False):
                """Closure units for one LN chunk: 8 stats, 1 rows, 8 apply."""
                st = {}
                units = []

                def stats_kt(kt):
                    def go():
                        if kt == 0:
                            st["mu"] = lnps.tile([1, CH], f32, tag="stat",
                                                 bufs=2, name="mu")
                            st["ss"] = lnps.tile([1, CH], f32, tag="stat",
                                                 bufs=2, name="ss")
                            st["xbfs"] = []
                        src, needs_dma = src_fn(kt, ch)
                        if needs_dma:
                            xt = lnsb.tile([128, CH], f32, tag="xin", bufs=4)
                            nc.sync.dma_start(xt[:], src)
                            src = xt
                        if src_bf:
                            xbf = src
                        else:
                            xbf = lnsb.tile([128, CH], bf16, tag="xbf",
                                            bufs=9)
                            nc.vector.tensor_copy(xbf[:], src[:])
                        st["xbfs"].append(xbf)
                        sq = lnsb.tile([128, CH], bf16, tag="sq", bufs=2)
                        nc.scalar.square(sq[:], xbf[:])
                        nc.tensor.matmul(
                            st["mu"][:], ones_col_bf[:], xbf[:],
                            start=(kt == 0), stop=(kt == KT - 1),
                            skip_group_check=skipg)
                        nc.tensor.matmul(
                            st["ss"][:], ones_col_bf[:], sq[:],
                            start=(kt == 0), stop=(kt == KT - 1),
                            skip_group_check=skipg)
                    return go

                def rows():
                    musq = lnsb.tile([1, CH], f32, tag="row", bufs=4)
                    nc.scalar.activation(musq[:], st["mu"][:], AF.Square,
                                         scale=1.0 / E)
                    varq = lnsb.tile([1, CH], f32, tag="row", bufs=4)
                    nc.vector.scalar_tensor_tensor(
                        varq[:], st["ss"][:], 1.0 / E, musq[:],
                        ALU.mult, ALU.subtract)
                    sd = lnsb.tile([1, CH], f32, tag="row", bufs=4)
                    nc.scalar.activation(sd[:], varq[:], AF.Sqrt,
                                         bias=eps_t[:])
                    a_row = lnsb.tile([1, CH], f32, tag="row", bufs=4)
                    nc.vector.reciprocal_approx_fast(out=a_row[:], in_=sd[:])
                    a_bf = lnsb.tile([1, CH], bf16, tag="rbf", bufs=3)
                    nc.vector.tensor_copy(a_bf[:], a_row[:])
                    mu_bf = lnsb.tile([1, CH], bf16, tag="rbf", bufs=3)
                    nc.scalar.mul(mu_bf[:], st["mu"][:], 1.0 / E)
                    A_ps = lnps.tile([128, CH], f32, tag="ac", bufs=4,
                                     name="A_ps")
                    nc.tensor.matmul(A_ps[:], ones_row_bf128[:], a_bf[:],
                                     start=True, stop=True)
                    M_ps = lnps.tile([128, CH], f32, tag="ac", bufs=4,
                                     name="M_ps")
                    nc.tensor.matmul(M_ps[:], ones_row_bf128[:], mu_bf[:],
                                     start=True, stop=True)
                    st["a_bc"] = A_ps
                    st["mu_bc"] = M_ps

                def apply_kt(kt):
                    def go():
                        t1 = lnsb.tile([128, CH], f32, tag="t1", bufs=2)
                        nc.vector.tensor_sub(t1[:], st["xbfs"][kt][:],
                                             st["mu_bc"][:])
                        t2 = lnsb.tile([128, CH], f32, tag="t2", bufs=2)
                        nc.vector.tensor_mul(t2[:], t1[:], st["a_bc"][:])
                        nc.vector.tensor_scalar(
                            dst_fn(kt, ch), t2[:],
                            g_row[:, kt, 0:1], b_col[:, kt, 0:1],
                            ALU.mult, ALU.add)
                    return go

                for kt in range(KT):
                    units.append(stats_kt(kt))
                units.append(rows)
                for kt in range(KT):
                    units.append(apply_kt(kt))
                return units

            def roundrobin(a_units, b_units):
                """Interleave two unit lists proportionally, run all."""
                na, nb = len(a_units), len(b_units)
                ai = bi = 0
                for i in range(na + nb):
                    if ai * nb <= bi * na and ai < na:
                        a_units[ai]()
                        ai += 1
                    elif bi < nb:
                        b_units[bi]()
                        bi += 1
                    else:
                        a_units[ai]()
                        ai += 1

            # ---------------- Phase 1: LN1 pools ----------------
            ln_ctx = ExitStack()
            lnps1 = ln_ctx.enter_context(
                tc.tile_pool(name="lnp", bufs=2, space="PSUM"))
            lnsb1 = ln_ctx.enter_context(tc.tile_pool(name="lns", bufs=3))

            def x_src(kt, ch):
                return (xT_d[kt * 128:(kt + 1) * 128,
                             ch * CH:(ch + 1) * CH], True)

            def ln1_ch(ch):
                return ln_units(ln1_slice, x_src, ch, ln1_gr, ln1_b, False,
                                lnps1, lnsb1, skipg=True)

            # ---------------- Phase 2: QKV units ----------------
            def qk_mms(ps, p8, ml, nq):
                for kp in range(4):
                    nc.tensor.matmul(
                        ps[:], p8[:, 2 * kp:2 * kp + 2,
                                  ml * 128:(ml + 1) * 128],
                        ln1_dr(kp, nq),
                        start=(kp == 0), stop=(kp == 3),
                        perf_mode=DR)

            def q_unit(panels, ml, mt, nq):
                ps = gps.tile([128, CH], f32, tag="ps")
                qk_mms(ps, panels, ml, nq)
                nc.vector.tensor_scalar(
                    qT[:, mt, nq * CH:(nq + 1) * CH], ps[:],
                    1.0 / 32, b_q[:, mt, 0:1], ALU.mult, ALU.add)

            def k_unit(panels, ml, mt, nq):
                ps = gps.tile([128, CH], f32, tag="ps")
                qk_mms(ps, panels, ml, nq)
                nc.vector.tensor_scalar(
                    kT[:, mt, nq * CH:(nq + 1) * CH], ps[:],
                    1.0 / 32, None, ALU.mult)

            def v_bias(g):
                # g indexes a 256-wide (4-head) group of V features
                bvt = gps.tile([128, CH], f32, tag="ps")
                bv_bc = bvt[:, 0:256]
                nc.tensor.matmul(bv_bc, ones_row_bf128[:],
                                 b_v[:, g * 256:(g + 1) * 256],
                                 start=True, stop=True)
                bv_sb = qksb.tile([128, 256], bf16, tag="bvs", bufs=2)
                nc.vector.tensor_copy(bv_sb[:], bv_bc)
                return bv_sb.rearrange("p (h w) -> p h w", h=4)

            def v_unit(p8, bv3, g, mv):
                pst = gps.tile([128, CH], f32, tag="ps")
                ps = pst[:, 0:256]
                for kp in range(4):
                    nc.tensor.matmul(
                        ps,
                        ln1_dr(kp, mv // 4)[:, :, (mv % 4) * 128:
                                            (mv % 4) * 128 + 128],
                        p8[:, 2 * kp:2 * kp + 2, :],
                        start=(kp == 0), stop=(kp == 3),
                        perf_mode=DR)
                ps3 = ps.rearrange("p (h w) -> p h w", h=4)
                nc.vector.scalar_tensor_tensor(
                    v4[:, mv, g * 4:(g + 1) * 4, 0:64], ps3[:],
                    1.0 / 32, bv3[:], ALU.mult, ALU.add)
                nc.vector.memset(v4[:, mv, g * 4:(g + 1) * 4, 64:65], 1.0)

            def qkv_phase(nqs, mvs, qn):
                """Emit a list of closures for K,V,Q over given positions."""
                units = []
                holder = {}

                def dma_panel(key, cols):
                    def go():
                        p8 = wqk.tile([128, KT, 256], fp8, tag="w8", bufs=2,
                                      name="p8")
                        nc.sync.dma_start(p8[:], w8a_r[:, :, cols:cols + 256])
                        holder[key] = p8
                    return go

                for g in range(4):  # K panels (2 m-tiles each)
                    units.append(dma_panel(("k", g), E + g * 256))
                    for ml in range(2):
                        for nq in nqs:
                            units.append(lambda g=g, ml=ml, nq=nq: k_unit(
                                holder[("k", g)], ml, g * 2 + ml, nq))
                for g in range(4):  # V panels (4 heads each)
                    units.append(dma_panel(("v", g), 2 * E + g * 256))

                    def vb(g=g):
                        holder[("bv", g)] = v_bias(g)
                    units.append(vb)
                    for mv in mvs:
                        units.append(lambda g=g, mv=mv: v_unit(
                            holder[("v", g)], holder[("bv", g)], g, mv))
                if qn is not None:
                    for g in range(4):  # Q panels (2 m-tiles each)
                        units.append(dma_panel(("q", g), g * 256))
                        for ml in range(2):
                            units.append(lambda g=g, ml=ml: q_unit(
                                holder[("q", g)], ml, g * 2 + ml, qn))
                return units

            for u in ln1_ch(0):
                u()
            roundrobin(qkv_phase((0,), (0, 1, 2, 3), 0),
                       ln1_ch(2) + ln1_ch(1) + ln1_ch(3))
            for u in qkv_phase((2,), (8, 9, 10, 11), None):
                u()
            ln_ctx.close()
            fill_a = qkv_phase((1, 3), (4, 5, 6, 7, 12, 13, 14, 15), 1)

            # ---------------- Phase 3: attention ----------------
            atps_ctx = ExitStack()
            atps = atps_ctx.enter_context(
                tc.tile_pool(name="atps", bufs=1, space="PSUM"))
            pairs_a = [((0, 1), ("diag", 0)), ((2, 3), ("diag", 1)),
                       ((8, 9), ("drv", "A", 0)), ((10, 11), ("drv", "A", 0))]
            pairs_b = [((4, 5), ("diag", 0)), ((6, 7), ("diag", 1)),
                       ((0, 1), ("drv", "B", 0)), ((2, 3), ("drv", "B", 0)),
                       ((8, 9), ("drv", "B", 1)), ((10, 11), ("drv", "B", 1)),
                       ((12, 13), ("drv", "B", 2)), ((14, 15), ("drv", "B", 2))]

            def attn_slot(hp, slot, pairs, after_first=None):
                qc = slice(slot * CH, (slot + 1) * CH)
                outs = [atps.tile([65, CH], f32, tag=f"av{h01}", bufs=1,
                                  name=f"av{h01}")
                        for h01 in range(2)]
                last = len(pairs) - 1
                for pi, ((t0, t1), mk) in enumerate(pairs):
                    sps = [atps.tile([128, 2, CH], f32, tag="s", bufs=2,
                                     name=f"s{h01}")
                           for h01 in range(2)]
                    for tw, t in enumerate((t0, t1)):
                        for h01 in range(2):
                            ro = h01 * 64
                            nc.tensor.matmul(
                                sps[h01][:, tw, :],
                                kT[ro:ro + 64, hp, t * 128:(t + 1) * 128],
                                qT[ro:ro + 64, hp, qc],
                                start=True, stop=True)
                    if pi == 0 and after_first is not None:
                        after_first()
                    for h01 in range(2):
                        es = atsb.tile([128, 2, CH], bf16, tag="es", bufs=3)
                        if mk[0] == "diag":
                            nc.scalar.activation(es[:], sps[h01][:], AF.Exp)
                            nc.vector.tensor_mul(
                                es[:], es[:],
                                dmask[:, 2 * mk[1]:2 * mk[1] + 2, :])
                        else:
                            sc = sA_s if mk[1] == "A" else sB_s
                            bi = sA_b if mk[1] == "A" else sB_b
                            idx = mk[2]
                            nc.scalar.activation(
                                es[:], sps[h01][:], AF.Exp,
                                bias=bi[:, idx:idx + 1],
                                scale=sc[:, idx:idx + 1])
                        h = 2 * hp + h01
                        for tw, t in enumerate((t0, t1)):
                            nc.tensor.matmul(
                                outs[h01][:], v_aug[:, t, h * 65:(h + 1) * 65],
                                es[:, tw, :],
                                start=(pi == 0 and tw == 0),
                                stop=(pi == last and tw == 1),
                                skip_group_check=True)
                def finalize():
                    for h01 in range(2):
                        ro = h01 * 64
                        d_bf = atsb.tile([1, CH], bf16, tag="dn", bufs=2)
                        nc.vector.tensor_copy(d_bf[:], outs[h01][64:65, :])
                        bc = atps.tile([128, 2, CH], f32, tag="s", bufs=2)
                        nc.tensor.matmul(bc[0:64, 0, :], ones_row_bf[:],
                                         d_bf[:], start=True, stop=True)
                        rec = atsb.tile([64, CH], f32, tag="rc", bufs=2)
                        nc.vector.reciprocal_approx_fast(
                            out=rec[:], in_=bc[0:64, 0, :])
                        nc.vector.tensor_mul(
                            attnT[ro:ro + 64, hp, qc],
                            outs[h01][0:64, :], rec[:])
                return finalize

            p2a_ctx.close()  # ln1_even released (only odd needed by filler)

            fi = 0
            pend = None
            for hp in range(8):
                pend = attn_slot(hp, 0, pairs_a, after_first=pend)
                take = (len(fill_a) * (hp + 1)) // 8
                while fi < take:
                    fill_a[fi]()
                    fi += 1
            pend()
            p2_ctx.close()

            # ---------------- attention slot B + ap-A filler ----------------
            def ap_unit(m, nq):
                panel = app.tile([128, KT, 128], bf16, tag="w", bufs=2)
                nc.sync.dma_start(panel[:], w_ap_r[:, :, m * 128:(m + 1) * 128])
                ps = gps.tile([128, CH], f32, tag="ps")
                for kt in range(KT):
                    nc.tensor.matmul(
                        ps[:], panel[:, kt, :],
                        attnT[:, kt, nq * CH:(nq + 1) * CH],
                        start=(kt == 0), stop=(kt == KT - 1))
                xq = gsb.tile([128, CH], f32, tag="xq", bufs=2)
                nc.sync.dma_start(
                    xq[:], xT_d[m * 128:(m + 1) * 128,
                                nq * CH:(nq + 1) * CH])
                nc.vector.scalar_tensor_tensor(
                    x2T[:, m, nq * CH:(nq + 1) * CH], ps[:],
                    b_ap[:, m, 0:1], xq[:], ALU.add, ALU.add)

            ap_sched = [0, 1, 1, 1, 1, 1, 1, 2]
            ai = 0
            pend = None
            for hp in range(8):
                pend = attn_slot(hp, 1, pairs_b, after_first=pend)
                for _ in range(ap_sched[hp]):
                    ap_unit(ai, 0)
                    ai += 1
            pend()

            atps_ctx.close()
            atsb_ctx.close()
            qkv_ctx.close()

            # -------- ap-B interleaved with LN2 chunk 0 --------
            ln2_ctx = ExitStack()
            lnps2 = ln2_ctx.enter_context(
                tc.tile_pool(name="lnp2", bufs=2, space="PSUM"))
            lnsb2 = ln2_ctx.enter_context(tc.tile_pool(name="lns2", bufs=3))

            def h2_dst(kt, ch):
                return h2B[:, kt, ch * CH:(ch + 1) * CH]

            def x2_src(kt, ch):
                return (x2T[:, kt, ch * CH:(ch + 1) * CH], False)

            def ln2_ch(ch):
                return ln_units(h2_dst, x2_src, ch, ln2_gr, ln2_b, True,
                                lnps2, lnsb2, skipg=True)

            apb = [(lambda m=m: ap_unit(m, 1)) for m in range(KT)]
            roundrobin(ln2_ch(0), apb)
            for u in ln2_ch(1):
                u()
            ln2_ctx.close()
            attn_ctx.close()

            # ---------------- FC, proj ----------------
            tail = ExitStack()
            gp = tail.enter_context(tc.tile_pool(name="gp", bufs=1))
            gT = gp.tile([128, 32, CH], bf16)
            wfcp = tail.enter_context(tc.tile_pool(name="wfcp", bufs=2))

            def fc_unit(mg, nq):
                panel = wfcp.tile([128, KT, 256], bf16, tag="w", bufs=2)
                nc.sync.dma_start(panel[:],
                                  w_fc_r[:, :, mg * 256:(mg + 1) * 256])
                sl = slice(nq * CH, (nq + 1) * CH)
                for mm in range(2):
                    mt = mg * 2 + mm
                    ps = gps.tile([128, CH], f32, tag="ps")
                    for kt in range(KT):
                        nc.tensor.matmul(
                            ps[:], panel[:, kt, mm * 128:(mm + 1) * 128],
                            h2B[:, kt, sl],
                            start=(kt == 0), stop=(kt == KT - 1))
                    nc.scalar.activation(
                        gT[:, mt, :], ps[:],
                        AF.Gelu, bias=b_fc[:, mt, 0:1])

            def proj_slot(nq):
                for half in range(2):
                    with tc.tile_pool(name="wprp", bufs=3) as wprp, \
                         tc.tile_pool(name="prps", bufs=4, space="PSUM") as prps:
                        pss = [prps.tile([128, CH], f32, tag="ps",
                                         name=f"prps{m}", bufs=4)
                               for m in range(4)]
                        hs = slice(half * CH, (half + 1) * CH)
                        for kt in range(32):
                            panel = wprp.tile([128, CH], bf16, tag="w")
                            nc.sync.dma_start(
                                panel[:], w_pr_d[kt * 128:(kt + 1) * 128, hs])
                            for m in range(4):
                                nc.tensor.matmul(
                                    pss[m][:],
                                    panel[:, m * 128:(m + 1) * 128],
                                    gT[:, kt, :],
                                    start=(kt == 0), stop=(kt == 31),
                                    skip_group_check=True)
                        for m in range(4):
                            gm = half * 4 + m
                            ot = gsb.tile([128, CH], f32, tag="ot", bufs=2)
                            nc.vector.scalar_tensor_tensor(
                                ot[:], pss[m][:], b_pr[:, gm, 0:1],
                                x2T[:, gm, nq * CH:(nq + 1) * CH],
                                ALU.add, ALU.add)
                            nc.sync.dma_start(
                                out_d[gm * 128:(gm + 1) * 128,
                                      nq * CH:(nq + 1) * CH], ot[:])

            for mg in range(16):
                fc_unit(mg, 0)
            proj_slot(0)
            for mg in range(16):
                fc_unit(mg, 1)
            proj_slot(1)
            tail.close()
            gemm_ps_ctx.close()

    nc.compile()
    return nc


def _host_prep(inputs):
    """Build the 8 per-core input maps."""
    x = np.asarray(inputs["x"], np.float32)
    w_attn = np.asarray(inputs["w_attn"], np.float32).copy()
    w_attn[:, :E] *= 0.125  # fold 1/sqrt(head_dim) into Q
    b_attn = np.asarray(inputs["b_attn"], np.float32).copy()
    b_attn[:E] *= 0.125
    w8a = np.ascontiguousarray(np.clip(w_attn * 32.0, -240, 240)
                               .astype(ml_dtypes.float8_e4m3))
    b_q = np.ascontiguousarray(b_attn[:E].reshape(E, 1))
    b_v = np.ascontiguousarray(b_attn[2 * E:].reshape(1, E).astype(BF))
    w_ap_bf = np.ascontiguousarray(np.asarray(inputs["w_attnproj"], np.float32).astype(BF))
    w_fc_bf = np.ascontiguousarray(np.asarray(inputs["w_fc"], np.float32).astype(BF))
    w_pr_bf = np.ascontiguousarray(np.asarray(inputs["w_proj"], np.float32).astype(BF))
    col = lambda v: np.ascontiguousarray(np.asarray(v, np.float32).reshape(-1, 1))
    b_ap = col(inputs["b_attnproj"])
    b_fc = col(inputs["b_fc"])
    b_pr = col(inputs["b_proj"])
    ln1_g = col(inputs["ln1_g"]); ln1_b = col(inputs["ln1_b"])
    ln2_g = col(inputs["ln2_g"]); ln2_b = col(inputs["ln2_b"])

    # multiplicative diagonal masks: dmask01[r][p, j] = (j >= r*128+p)
    j = np.arange(CH)[None, :]
    p = np.arange(128)[:, None]
    dmask01 = np.stack([(j >= r * 128 + p).astype(np.float32)
                        for r in range(4)]).astype(BF)
    dmask01 = np.ascontiguousarray(dmask01)

    ON = (1.0, 0.0)
    OFF = (0.0, NEG)
    in_maps = []
    perms = []
    for core in range(8):
        b = core // 2
        z = core % 2
        blocks = [0, 3, 1, 2] if z == 0 else [1, 2, 0, 3]
        perms.append(blocks)
        cols = np.concatenate([np.arange(c * CH, (c + 1) * CH) for c in blocks])
        xT = np.ascontiguousarray(x[b].T[:, cols])
        sa = ON if blocks[2] < blocks[0] else OFF
        sbs = [ON if blocks[i] < blocks[1] else OFF for i in (0, 2, 3)]
        f = np.float32
        in_maps.append({
            "xT": xT,
            "w8_attn": w8a, "b_q": b_q, "b_v": b_v,
            "w_ap": w_ap_bf, "b_ap": b_ap,
            "ln1_g": ln1_g, "ln1_b": ln1_b, "ln2_g": ln2_g, "ln2_b": ln2_b,
            "w_fc": w_fc_bf, "b_fc": b_fc,
            "w_proj": w_pr_bf, "b_proj": b_pr,
            "dmask01": dmask01,
            "sA_scale": np.full((128, 1), sa[0], f),
            "sA_bias": np.full((128, 1), sa[1], f),
            "sB_scale": np.ascontiguousarray(
                np.tile(np.array([[s for s, _ in sbs]], f), (128, 1))),
            "sB_bias": np.ascontiguousarray(
                np.tile(np.array([[bb for _, bb in sbs]], f), (128, 1))),
        })
    return in_maps, perms


def _run(inputs, trace=False):
    from concourse.bass_utils import run_bass_kernel_spmd

    if "nc" not in _CACHE:
        _CACHE["nc"] = _build_program()
    nc = _CACHE["nc"]
    in_maps, perms = _host_prep(inputs)
    res = run_bass_kernel_spmd(nc, in_maps, list(range(8)), trace=trace)
    x = np.asarray(inputs["x"], np.float32)
    out = np.empty_like(x)
    for core in range(8):
        b = core // 2
        blocks = perms[core]
        oT = res.results[core]["outT"]
        cA, cB = blocks[0], blocks[1]
        out[b, cA * CH:(cA + 1) * CH, :] = oT[:, 0:CH].T
        out[b, cB * CH:(cB + 1) * CH, :] = oT[:, CH:2 * CH].T
    return out, res


def kernel(**inputs) -> np.ndarray:
    out, _ = _run(inputs, trace=False)
    return out


# revision 29
# speedup vs baseline: 1.1542x; 1.1542x over previous
"""GPT block (LN -> causal MHA -> LN -> MLP) on 8 TRN2 NeuronCores.

Sharding: each core owns one (batch, zig-zag query-chunk-pair); B=4 x 2
pairs = 8 cores. K/V are recomputed per core for the full sequence (no
collectives); per-core causality uses a block permutation of the
sequence with static diagonal masks and per-core exp scale/bias inputs.

Engine/pipeline structure:
  - LN: bf16 stats via ones-column matmuls, scalar-engine row chains,
    reciprocal_approx_fast, broadcast mu/rstd once per chunk, 3-op DVE
    apply. LN1 chunk 0 first, remaining chunks interleaved with QKV.
  - GEMM dtypes: QKV runs fp8e4 DoubleRow with x32-scaled weights
    (4 matmuls per 1024-contract instead of 8; the 1/32 is folded into
    the bias/copy ops). FC / proj / attnproj stay bf16 -- on this part
    a matmul instruction costs the same regardless of dtype, so fp8
    only pays where it halves the instruction count. K bias dropped
    (softmax-invariant), 1/sqrt(d) folded into Q.
  - Attention: q/k stored fp8e4; 2-head row-packed score matmuls, exp
    batched over kv-tile pairs ([128,1024] ACT), multiplicative bf16
    diagonal masks after exp, softmax denominator via ones-column in
    the AV matmul + broadcast + reciprocal_approx_fast; the per-head
    finalize is deferred into the next head's score phase.
  - Interleave for HAM warmth: QKV for perm positions {0,2} first,
    attention slot A with remaining QKV as PE filler, attention slot B
    with attnproj-A as filler, then attnproj-B || LN2, FC, and a
    two-half-pass proj tail.
"""

import numpy as np
import ml_dtypes

BF = ml_dtypes.bfloat16

E = 1024          # embedding
T = 2048          # sequence
B = 4             # batch
NH = 16           # heads
D = 64            # head dim
HID = 4096        # mlp hidden
KT = E // 128     # k-tiles over embedding (8)
CH = 512          # chunk rows
NEG = -1.0e9
EPS = 1e-5

_CACHE = {}


def _build_program():
    import concourse.bass as bass
    import concourse.tile as tile
    from concourse import bacc, mybir

    f32 = mybir.dt.float32
    f32r = mybir.dt.float32r
    bf16 = mybir.dt.bfloat16
    fp8 = mybir.dt.float8e4
    DR = mybir.MatmulPerfMode.DoubleRow
    AF = mybir.ActivationFunctionType
    ALU = mybir.AluOpType

    nc = bacc.Bacc()

    xT_d = nc.declare_dram_parameter("xT", [E, T], f32, isOutput=False)
    w8a_d = nc.declare_dram_parameter("w8_attn", [E, 3 * E], fp8, isOutput=False)
    b_q_d = nc.declare_dram_parameter("b_q", [E, 1], f32, isOutput=False)
    b_v_d = nc.declare_dram_parameter("b_v", [1, E], bf16, isOutput=False)
    w_ap_d = nc.declare_dram_parameter("w_ap", [E, E], bf16, isOutput=False)
    b_ap_d = nc.declare_dram_parameter("b_ap", [E, 1], f32, isOutput=False)
    ln1_g_d = nc.declare_dram_parameter("ln1_g", [E, 1], f32, isOutput=False)
    ln1_b_d = nc.declare_dram_parameter("ln1_b", [E, 1], f32, isOutput=False)
    ln2_g_d = nc.declare_dram_parameter("ln2_g", [E, 1], f32, isOutput=False)
    ln2_b_d = nc.declare_dram_parameter("ln2_b", [E, 1], f32, isOutput=False)
    w_fc_d = nc.declare_dram_parameter("w_fc", [E, HID], bf16, isOutput=False)
    b_fc_d = nc.declare_dram_parameter("b_fc", [HID, 1], f32, isOutput=False)
    w_pr_d = nc.declare_dram_parameter("w_proj", [HID, E], bf16, isOutput=False)
    b_pr_d = nc.declare_dram_parameter("b_proj", [E, 1], f32, isOutput=False)
    dmask_d = nc.declare_dram_parameter("dmask01", [4, 128, CH], bf16, isOutput=False)
    sA_s_d = nc.declare_dram_parameter("sA_scale", [128, 1], f32, isOutput=False)
    sA_b_d = nc.declare_dram_parameter("sA_bias", [128, 1], f32, isOutput=False)
    sB_s_d = nc.declare_dram_parameter("sB_scale", [128, 3], f32, isOutput=False)
    sB_b_d = nc.declare_dram_parameter("sB_bias", [128, 3], f32, isOutput=False)
    out_d = nc.declare_dram_parameter("outT", [E, 2 * CH], f32, isOutput=True)

    w8a_r = w8a_d.rearrange("(k p) n -> p k n", p=128)
    w_ap_r = w_ap_d.rearrange("(k p) n -> p k n", p=128)
    w_fc_r = w_fc_d.rearrange("(k p) n -> p k n", p=128)

    with tile.TileContext(nc) as tc:
        from contextlib import ExitStack

        stack = ExitStack()
        with stack:
            const = stack.enter_context(tc.tile_pool(name="const", bufs=1))

            ones_col_f = const.tile([128, 1], f32)
            nc.vector.memset(ones_col_f[:], 1.0)
            ones_col_bf = const.tile([128, 1], bf16)
            nc.vector.memset(ones_col_bf[:], 1.0)
            ones_row_bf = const.tile([1, 64], bf16)
            nc.vector.memset(ones_row_bf[:], 1.0)
            ones_row_f = const.tile([1, 128], f32)
            nc.vector.memset(ones_row_f[:], 1.0)
            eps_t = const.tile([1, 1], f32)
            nc.vector.memset(eps_t[:], EPS)

            dmask = const.tile([128, 4, CH], bf16)
            nc.sync.dma_start(dmask[:], dmask_d.rearrange("v p n -> p v n"))
            sA_s = const.tile([128, 1], f32)
            nc.sync.dma_start(sA_s[:], sA_s_d[:])
            sA_b = const.tile([128, 1], f32)
            nc.sync.dma_start(sA_b[:], sA_b_d[:])
            sB_s = const.tile([128, 3], f32)
            nc.sync.dma_start(sB_s[:], sB_s_d[:])
            sB_b = const.tile([128, 3], f32)
            nc.sync.dma_start(sB_b[:], sB_b_d[:])

            ln1_gr = const.tile([128, KT, 1], f32)
            nc.sync.dma_start(ln1_gr[:], ln1_g_d.rearrange("(k p) o -> p k o", p=128))
            ln2_gr = const.tile([128, KT, 1], f32)
            nc.sync.dma_start(ln2_gr[:], ln2_g_d.rearrange("(k p) o -> p k o", p=128))
            ones_row_bf128 = const.tile([1, 128], bf16)
            nc.vector.memset(ones_row_bf128[:], 1.0)
            ln1_b = const.tile([128, KT, 1], f32)
            nc.sync.dma_start(ln1_b[:], ln1_b_d.rearrange("(k p) o -> p k o", p=128))
            ln2_b = const.tile([128, KT, 1], f32)
            nc.sync.dma_start(ln2_b[:], ln2_b_d.rearrange("(k p) o -> p k o", p=128))
            b_q = const.tile([128, 8, 1], f32)
            nc.sync.dma_start(b_q[:], b_q_d.rearrange("(k p) o -> p k o", p=128))
            b_v = const.tile([1, E], bf16)
            nc.sync.dma_start(b_v[:], b_v_d[:])
            b_ap = const.tile([128, KT, 1], f32)
            nc.sync.dma_start(b_ap[:], b_ap_d.rearrange("(k p) o -> p k o", p=128))
            b_fc = const.tile([128, 32, 1], f32)
            nc.sync.dma_start(b_fc[:], b_fc_d.rearrange("(k p) o -> p k o", p=128))
            b_pr = const.tile([128, KT, 1], f32)
            nc.sync.dma_start(b_pr[:], b_pr_d.rearrange("(k p) o -> p k o", p=128))

            warm = const.tile([128, CH], bf16)
            nc.vector.memset(warm[:], 0.0)

            # ---- long-lived pools (open order = reverse close order) ----
            gsb = stack.enter_context(tc.tile_pool(name="gsb", bufs=2))
            x2p = stack.enter_context(tc.tile_pool(name="x2p", bufs=1))
            x2T = x2p.tile([128, KT, 2 * CH], bf16)
            h2p = stack.enter_context(tc.tile_pool(name="h2p", bufs=1))
            h2B = h2p.tile([128, KT, 2 * CH], bf16)
            app = stack.enter_context(tc.tile_pool(name="app", bufs=2))
            attn_ctx = ExitStack()
            attnp = attn_ctx.enter_context(tc.tile_pool(name="attnp", bufs=1))
            attnT = attnp.tile([128, KT, 2 * CH], bf16)

            # PSUM: gemm pool at stack bottom (lives past attention)
            gemm_ps_ctx = ExitStack()
            gps = gemm_ps_ctx.enter_context(
                tc.tile_pool(name="gps", bufs=2, space="PSUM"))

            qkv_ctx = ExitStack()
            qkvp = qkv_ctx.enter_context(tc.tile_pool(name="qkvp", bufs=1))
            qT = qkvp.tile([128, KT, 2 * CH], fp8)
            kT = qkvp.tile([128, KT, T], fp8)
            v_aug = qkvp.tile([128, 16, NH * 65], bf16)
            v4 = v_aug.rearrange("p m (h w) -> p m h w", h=NH)

            atsb_ctx = ExitStack()
            atsb = atsb_ctx.enter_context(tc.tile_pool(name="atsb", bufs=4))

            # pools that die at end of attention slot A
            p2_ctx = ExitStack()
            ln1o_p = p2_ctx.enter_context(tc.tile_pool(name="ln1o", bufs=1))
            ln1_odd = ln1o_p.tile([128, KT, 2, CH], fp8)   # perm pos {1,3}
            wqk = p2_ctx.enter_context(tc.tile_pool(name="wqk", bufs=2))
            qksb = p2_ctx.enter_context(tc.tile_pool(name="qksb", bufs=2))

            p2a_ctx = ExitStack()
            ln1e_p = p2a_ctx.enter_context(tc.tile_pool(name="ln1e", bufs=1))
            ln1_even = ln1e_p.tile([128, KT, 2, CH], fp8)  # perm pos {0,2}

            def ln1_slice(kt, ch):
                t = ln1_even if ch in (0, 2) else ln1_odd
                return t[:, kt, ch // 2 if ch in (0, 2) else (ch - 1) // 2, :]

            def ln1_dr(kp, ch):
                t = ln1_even if ch in (0, 2) else ln1_odd
                return t[:, 2 * kp:2 * kp + 2,
                         ch // 2 if ch in (0, 2) else (ch - 1) // 2, :]

            def ln_units(dst_fn, src_fn, ch, g_row, b_col, src_bf,
                         lnps, lnsb, skipg=False):
                """Closure units for one LN chunk: 8 stats, 1 rows, 8 apply."""
                st = {}
                units = []

                def stats_kt(kt):
                    def go():
                        if kt == 0:
                            st["mu"] = lnps.tile([1, CH], f32, tag="stat",
                                                 bufs=2, name="mu")
                            st["ss"] = lnps.tile([1, CH], f32, tag="stat",
                                                 bufs=2, name="ss")
                            st["xbfs"] = []
                        src, needs_dma = src_fn(kt, ch)
                        if needs_dma:
                            xt = lnsb.tile([128, CH], f32, tag="xin", bufs=4)
                            nc.sync.dma_start(xt[:], src)
                            src = xt
                        if src_bf:
                            xbf = src
                        else:
                            xbf = lnsb.tile([128, CH], bf16, tag="xbf",
                                            bufs=9)
                            nc.vector.tensor_copy(xbf[:], src[:])
                        st["xbfs"].append(xbf)
                        sq = lnsb.tile([128, CH], bf16, tag="sq", bufs=2)
                        nc.scalar.square(sq[:], xbf[:])
                        nc.tensor.matmul(
                            st["mu"][:], ones_col_bf[:], xbf[:],
                            start=(kt == 0), stop=(kt == KT - 1),
                            skip_group_check=skipg)
                        nc.tensor.matmul(
                            st["ss"][:], ones_col_bf[:], sq[:],
                            start=(kt == 0), stop=(kt == KT - 1),
                            skip_group_check=skipg)
                    return go

                def rows():
                    musq = lnsb.tile([1, CH], f32, tag="row", bufs=4)
                    nc.scalar.activation(musq[:], st["mu"][:], AF.Square,
                                         scale=1.0 / E)
                    varq = lnsb.tile([1, CH], f32, tag="row", bufs=4)
                    nc.vector.scalar_tensor_tensor(
                        varq[:], st["ss"][:], 1.0 / E, musq[:],
                        ALU.mult, ALU.subtract)
                    sd = lnsb.tile([1, CH], f32, tag="row", bufs=4)
                    nc.scalar.activation(sd[:], varq[:], AF.Sqrt,
                                         bias=eps_t[:])
                    a_row = lnsb.tile([1, CH], f32, tag="row", bufs=4)
                    nc.vector.reciprocal_approx_fast(out=a_row[:], in_=sd[:])
                    a_bf = lnsb.tile([1, CH], bf16, tag="rbf", bufs=3)
                    nc.vector.tensor_copy(a_bf[:], a_row[:])
                    mu_bf = lnsb.tile([1, CH], bf16, tag="rbf", bufs=3)
                    nc.scalar.mul(mu_bf[:], st["mu"][:], 1.0 / E)
                    A_ps = lnps.tile([128, CH], f32, tag="ac", bufs=4,
                                     name="A_ps")
                    nc.tensor.matmul(A_ps[:], ones_row_bf128[:], a_bf[:],
                                     start=True, stop=True)
                    M_ps = lnps.tile([128, CH], f32, tag="ac", bufs=4,
                                     name="M_ps")
                    nc.tensor.matmul(M_ps[:], ones_row_bf128[:], mu_bf[:],
                                     start=True, stop=True)
                    st["a_bc"] = A_ps
                    st["mu_bc"] = M_ps

                def apply_kt(kt):
                    def go():
                        t1 = lnsb.tile([128, CH], f32, tag="t1", bufs=2)
                        nc.vector.tensor_sub(t1[:], st["xbfs"][kt][:],
                                             st["mu_bc"][:])
                        t2 = lnsb.tile([128, CH], f32, tag="t2", bufs=2)
                        nc.vector.tensor_mul(t2[:], t1[:], st["a_bc"][:])
                        nc.vector.tensor_scalar(
                            dst_fn(kt, ch), t2[:],
                            g_row[:, kt, 0:1], b_col[:, kt, 0:1],
                            ALU.mult, ALU.add)
                    return go

                for kt in range(KT):
                    units.append(stats_kt(kt))
                units.append(rows)
                for kt in range(KT):
                    units.append(apply_kt(kt))
                return units

            def roundrobin(a_units, b_units):
                """Interleave two unit lists proportionally, run all."""
                na, nb = len(a_units), len(b_units)
                ai = bi = 0
                for i in range(na + nb):
                    if ai * nb <= bi * na and ai < na:
                        a_units[ai]()
                        ai += 1
                    elif bi < nb:
                        b_units[bi]()
                        bi += 1
                    else:
                        a_units[ai]()
                        ai += 1

            # Warmup: keep the PE (and its HAM clock gate) busy while the
            # first LN1 x-tiles stream in from HBM.
            with tc.tile_pool(name="wmps", bufs=1, space="PSUM") as wmps:
                wps = wmps.tile([1, CH], f32)
                for i in range(32):
                    nc.tensor.matmul(wps[:], ones_col_bf[:], warm[:],
                                     start=(i == 0), stop=(i == 31))

            # ---------------- Phase 1: LN1 pools ----------------
            ln_ctx = ExitStack()
            lnps1 = ln_ctx.enter_context(
                tc.tile_pool(name="lnp", bufs=2, space="PSUM"))
            lnsb1 = ln_ctx.enter_context(tc.tile_pool(name="lns", bufs=3))

            def x_src(kt, ch):
                return (xT_d[kt * 128:(kt + 1) * 128,
                             ch * CH:(ch + 1) * CH], True)

            def ln1_ch(ch):
                return ln_units(ln1_slice, x_src, ch, ln1_gr, ln1_b, False,
                                lnps1, lnsb1, skipg=True)

            # ---------------- Phase 2: QKV units ----------------
            def qk_mms(ps, p8, ml, nq):
                for kp in range(4):
                    nc.tensor.matmul(
                        ps[:], p8[:, 2 * kp:2 * kp + 2,
                                  ml * 128:(ml + 1) * 128],
                        ln1_dr(kp, nq),
                        start=(kp == 0), stop=(kp == 3),
                        perf_mode=DR)

            def q_unit(panels, ml, mt, nq):
                ps = gps.tile([128, CH], f32, tag="ps")
                qk_mms(ps, panels, ml, nq)
                nc.vector.tensor_scalar(
                    qT[:, mt, nq * CH:(nq + 1) * CH], ps[:],
                    1.0 / 32, b_q[:, mt, 0:1], ALU.mult, ALU.add)

            def k_unit(panels, ml, mt, nq):
                ps = gps.tile([128, CH], f32, tag="ps")
                qk_mms(ps, panels, ml, nq)
                nc.vector.tensor_scalar(
                    kT[:, mt, nq * CH:(nq + 1) * CH], ps[:],
                    1.0 / 32, None, ALU.mult)

            def v_bias(g):
                # g indexes a 256-wide (4-head) group of V features
                bvt = gps.tile([128, CH], f32, tag="ps")
                bv_bc = bvt[:, 0:256]
                nc.tensor.matmul(bv_bc, ones_row_bf128[:],
                                 b_v[:, g * 256:(g + 1) * 256],
                                 start=True, stop=True)
                bv_sb = qksb.tile([128, 256], bf16, tag="bvs", bufs=2)
                nc.vector.tensor_copy(bv_sb[:], bv_bc)
                return bv_sb.rearrange("p (h w) -> p h w", h=4)

            def v_unit(p8, bv3, g, mv):
                pst = gps.tile([128, CH], f32, tag="ps")
                ps = pst[:, 0:256]
                for kp in range(4):
                    nc.tensor.matmul(
                        ps,
                        ln1_dr(kp, mv // 4)[:, :, (mv % 4) * 128:
                                            (mv % 4) * 128 + 128],
                        p8[:, 2 * kp:2 * kp + 2, :],
                        start=(kp == 0), stop=(kp == 3),
                        perf_mode=DR)
                ps3 = ps.rearrange("p (h w) -> p h w", h=4)
                nc.vector.scalar_tensor_tensor(
                    v4[:, mv, g * 4:(g + 1) * 4, 0:64], ps3[:],
                    1.0 / 32, bv3[:], ALU.mult, ALU.add)
                nc.vector.memset(v4[:, mv, g * 4:(g + 1) * 4, 64:65], 1.0)

            def qkv_phase(nqs, mvs, qn):
                """Emit a list of closures for K,V,Q over given positions."""
                units = []
                holder = {}

                def dma_panel(key, cols):
                    def go():
                        p8 = wqk.tile([128, KT, 256], fp8, tag="w8", bufs=2,
                                      name="p8")
                        nc.sync.dma_start(p8[:], w8a_r[:, :, cols:cols + 256])
                        holder[key] = p8
                    return go

                for g in range(4):  # K panels (2 m-tiles each)
                    units.append(dma_panel(("k", g), E + g * 256))
                    for ml in range(2):
                        for nq in nqs:
                            units.append(lambda g=g, ml=ml, nq=nq: k_unit(
                                holder[("k", g)], ml, g * 2 + ml, nq))
                for g in range(4) if mvs else []:  # V panels
                    units.append(dma_panel(("v", g), 2 * E + g * 256))

                    def vb(g=g):
                        holder[("bv", g)] = v_bias(g)
                    units.append(vb)
                    for mv in mvs:
                        units.append(lambda g=g, mv=mv: v_unit(
                            holder[("v", g)], holder[("bv", g)], g, mv))
                if qn is not None:
                    for g in range(4):  # Q panels (2 m-tiles each)
                        units.append(dma_panel(("q", g), g * 256))
                        for ml in range(2):
                            units.append(lambda g=g, ml=ml: q_unit(
                                holder[("q", g)], ml, g * 2 + ml, qn))
                return units

            for u in ln1_ch(0):
                u()
            roundrobin(qkv_phase((0,), (0, 1, 2, 3), 0),
                       ln1_ch(2) + ln1_ch(1) + ln1_ch(3))
            for u in qkv_phase((2,), (8, 9, 10, 11), None):
                u()
            ln_ctx.close()
            fill_a = qkv_phase((1,), (4, 5, 6, 7, 12, 13, 14, 15), 1)
            fill_b = qkv_phase((3,), (), None)

            # ---------------- Phase 3: attention ----------------
            atps_ctx = ExitStack()
            atps = atps_ctx.enter_context(
                tc.tile_pool(name="atps", bufs=1, space="PSUM"))
            pairs_a = [((0, 1), ("diag", 0)), ((2, 3), ("diag", 1)),
                       ((8, 9), ("drv", "A", 0)), ((10, 11), ("drv", "A", 0))]
            pairs_b = [((4, 5), ("diag", 0)), ((6, 7), ("diag", 1)),
                       ((0, 1), ("drv", "B", 0)), ((2, 3), ("drv", "B", 0)),
                       ((8, 9), ("drv", "B", 1)), ((10, 11), ("drv", "B", 1)),
                       ((12, 13), ("drv", "B", 2)), ((14, 15), ("drv", "B", 2))]

            def attn_slot(hp, slot, pairs, after_first=None):
                qc = slice(slot * CH, (slot + 1) * CH)
                outs = [atps.tile([65, CH], f32, tag=f"av{h01}", bufs=1,
                                  name=f"av{h01}")
                        for h01 in range(2)]
                last = len(pairs) - 1
                for pi, ((t0, t1), mk) in enumerate(pairs):
                    sps = [atps.tile([128, 2, CH], f32, tag="s", bufs=2,
                                     name=f"s{h01}")
                           for h01 in range(2)]
                    for tw, t in enumerate((t0, t1)):
                        for h01 in range(2):
                            ro = h01 * 64
                            nc.tensor.matmul(
                                sps[h01][:, tw, :],
                                kT[ro:ro + 64, hp, t * 128:(t + 1) * 128],
                                qT[ro:ro + 64, hp, qc],
                                start=True, stop=True)
                    if pi == 0 and after_first is not None:
                        after_first()
                    for h01 in range(2):
                        es = atsb.tile([128, 2, CH], bf16, tag="es", bufs=3)
                        if mk[0] == "diag":
                            nc.scalar.activation(es[:], sps[h01][:], AF.Exp)
                            nc.vector.tensor_mul(
                                es[:], es[:],
                                dmask[:, 2 * mk[1]:2 * mk[1] + 2, :])
                        else:
                            sc = sA_s if mk[1] == "A" else sB_s
                            bi = sA_b if mk[1] == "A" else sB_b
                            idx = mk[2]
                            nc.scalar.activation(
                                es[:], sps[h01][:], AF.Exp,
                                bias=bi[:, idx:idx + 1],
                                scale=sc[:, idx:idx + 1])
                        h = 2 * hp + h01
                        for tw, t in enumerate((t0, t1)):
                            nc.tensor.matmul(
                                outs[h01][:], v_aug[:, t, h * 65:(h + 1) * 65],
                                es[:, tw, :],
                                start=(pi == 0 and tw == 0),
                                stop=(pi == last and tw == 1),
                                skip_group_check=True)
                def finalize():
                    for h01 in range(2):
                        ro = h01 * 64
                        d_bf = atsb.tile([1, CH], bf16, tag="dn", bufs=2)
                        nc.vector.tensor_copy(d_bf[:], outs[h01][64:65, :])
                        bc = atps.tile([128, 2, CH], f32, tag="s", bufs=2)
                        nc.tensor.matmul(bc[0:64, 0, :], ones_row_bf[:],
                                         d_bf[:], start=True, stop=True)
                        rec = atsb.tile([64, CH], f32, tag="rc", bufs=2)
                        nc.vector.reciprocal_approx_fast(
                            out=rec[:], in_=bc[0:64, 0, :])
                        nc.vector.tensor_mul(
                            attnT[ro:ro + 64, hp, qc],
                            outs[h01][0:64, :], rec[:])
                return finalize

            p2a_ctx.close()  # ln1_even released (only odd needed by filler)

            fi = 0
            pend = None
            for hp in range(8):
                pend = attn_slot(hp, 0, pairs_a, after_first=pend)
                take = (len(fill_a) * (hp + 1)) // 8
                while fi < take:
                    fill_a[fi]()
                    fi += 1
            pend()

            # ---------------- attention slot B + ap-A filler ----------------
            def ap_unit(m, nq):
                panel = app.tile([128, KT, 128], bf16, tag="w", bufs=2)
                nc.sync.dma_start(panel[:], w_ap_r[:, :, m * 128:(m + 1) * 128])
                ps = gps.tile([128, CH], f32, tag="ps")
                for kt in range(KT):
                    nc.tensor.matmul(
                        ps[:], panel[:, kt, :],
                        attnT[:, kt, nq * CH:(nq + 1) * CH],
                        start=(kt == 0), stop=(kt == KT - 1))
                xq = gsb.tile([128, CH], f32, tag="xq", bufs=2)
                nc.sync.dma_start(
                    xq[:], xT_d[m * 128:(m + 1) * 128,
                                nq * CH:(nq + 1) * CH])
                nc.vector.scalar_tensor_tensor(
                    x2T[:, m, nq * CH:(nq + 1) * CH], ps[:],
                    b_ap[:, m, 0:1], xq[:], ALU.add, ALU.add)

            ap_sched = [0, 1, 1, 1, 1, 1, 1, 2]
            # issue count into fill_b needed before slot hp's drv-B2 pairs
            fb_need = [2, 3, 5, 6, 8, 9, 11, 12]
            ai = 0
            pend = None
            state = {"bi": 0}

            def mk_after(prev_fin, hp):
                def go():
                    if prev_fin is not None:
                        prev_fin()
                    while state["bi"] < fb_need[hp]:
                        fill_b[state["bi"]]()
                        state["bi"] += 1
                return go

            for hp in range(8):
                pend = attn_slot(hp, 1, pairs_b,
                                 after_first=mk_after(pend, hp))
                for _ in range(ap_sched[hp]):
                    ap_unit(ai, 0)
                    ai += 1
            pend()

            atps_ctx.close()
            p2_ctx.close()
            atsb_ctx.close()
            qkv_ctx.close()

            # -------- ap-B interleaved with LN2 chunk 0 --------
            ln2_ctx = ExitStack()
            lnps2 = ln2_ctx.enter_context(
                tc.tile_pool(name="lnp2", bufs=2, space="PSUM"))
            lnsb2 = ln2_ctx.enter_context(tc.tile_pool(name="lns2", bufs=3))

            def h2_dst(kt, ch):
                return h2B[:, kt, ch * CH:(ch + 1) * CH]

            def x2_src(kt, ch):
                return (x2T[:, kt, ch * CH:(ch + 1) * CH], False)

            def ln2_ch(ch):
                return ln_units(h2_dst, x2_src, ch, ln2_gr, ln2_b, True,
                                lnps2, lnsb2, skipg=True)

            apb = [(lambda m=m: ap_unit(m, 1)) for m in range(KT)]
            roundrobin(ln2_ch(0), apb)
            for u in ln2_ch(1):
                u()
            ln2_ctx.close()
            attn_ctx.close()

            # ---------------- FC, proj ----------------
            tail = ExitStack()
            gp = tail.enter_context(tc.tile_pool(name="gp", bufs=1))
            gT = gp.tile([128, 32, CH], bf16)
            wfcp = tail.enter_context(tc.tile_pool(name="wfcp", bufs=2))

            def fc_unit(mg, nq):
                panel = wfcp.tile([128, KT, 256], bf16, tag="w", bufs=2)
                nc.sync.dma_start(panel[:],
                                  w_fc_r[:, :, mg * 256:(mg + 1) * 256])
                sl = slice(nq * CH, (nq + 1) * CH)
                for mm in range(2):
                    mt = mg * 2 + mm
                    ps = gps.tile([128, CH], f32, tag="ps")
                    for kt in range(KT):
                        nc.tensor.matmul(
                            ps[:], panel[:, kt, mm * 128:(mm + 1) * 128],
                            h2B[:, kt, sl],
                            start=(kt == 0), stop=(kt == KT - 1))
                    nc.scalar.activation(
                        gT[:, mt, :], ps[:],
                        AF.Gelu, bias=b_fc[:, mt, 0:1])

            def proj_slot(nq):
                for half in range(2):
                    with tc.tile_pool(name="wprp", bufs=3) as wprp, \
                         tc.tile_pool(name="prps", bufs=4, space="PSUM") as prps:
                        pss = [prps.tile([128, CH], f32, tag="ps",
                                         name=f"prps{m}", bufs=4)
                               for m in range(4)]
                        hs = slice(half * CH, (half + 1) * CH)
                        for kt in range(32):
                            panel = wprp.tile([128, CH], bf16, tag="w")
                            nc.sync.dma_start(
                                panel[:], w_pr_d[kt * 128:(kt + 1) * 128, hs])
                            for m in range(4):
                                nc.tensor.matmul(
                                    pss[m][:],
                                    panel[:, m * 128:(m + 1) * 128],
                                    gT[:, kt, :],
                                    start=(kt == 0), stop=(kt == 31),
                                    skip_group_check=True)
                        for m in range(4):
                            gm = half * 4 + m
                            ot = gsb.tile([128, CH], f32, tag="ot", bufs=2)
                            nc.vector.scalar_tensor_tensor(
                                ot[:], pss[m][:], b_pr[:, gm, 0:1],
                                x2T[:, gm, nq * CH:(nq + 1) * CH],
                                ALU.add, ALU.add)
                            nc.sync.dma_start(
                                out_d[gm * 128:(gm + 1) * 128,
                                      nq * CH:(nq + 1) * CH], ot[:])

            for mg in range(16):
                fc_unit(mg, 0)
            proj_slot(0)
            for mg in range(16):
                fc_unit(mg, 1)
            proj_slot(1)
            tail.close()
            gemm_ps_ctx.close()

    nc.compile()
    return nc


def _host_prep(inputs):
    """Build the 8 per-core input maps."""
    x = np.asarray(inputs["x"], np.float32)
    w_attn = np.asarray(inputs["w_attn"], np.float32).copy()
    w_attn[:, :E] *= 0.125  # fold 1/sqrt(head_dim) into Q
    b_attn = np.asarray(inputs["b_attn"], np.float32).copy()
    b_attn[:E] *= 0.125
    w8a = np.ascontiguousarray(np.clip(w_attn * 32.0, -240, 240)
                               .astype(ml_dtypes.float8_e4m3))
    b_q = np.ascontiguousarray(b_attn[:E].reshape(E, 1))
    b_v = np.ascontiguousarray(b_attn[2 * E:].reshape(1, E).astype(BF))
    w_ap_bf = np.ascontiguousarray(np.asarray(inputs["w_attnproj"], np.float32).astype(BF))
    w_fc_bf = np.ascontiguousarray(np.asarray(inputs["w_fc"], np.float32).astype(BF))
    w_pr_bf = np.ascontiguousarray(np.asarray(inputs["w_proj"], np.float32).astype(BF))
    col = lambda v: np.ascontiguousarray(np.asarray(v, np.float32).reshape(-1, 1))
    b_ap = col(inputs["b_attnproj"])
    b_fc = col(inputs["b_fc"])
    b_pr = col(inputs["b_proj"])
    ln1_g = col(inputs["ln1_g"]); ln1_b = col(inputs["ln1_b"])
    ln2_g = col(inputs["ln2_g"]); ln2_b = col(inputs["ln2_b"])

    # multiplicative diagonal masks: dmask01[r][p, j] = (j >= r*128+p)
    j = np.arange(CH)[None, :]
    p = np.arange(128)[:, None]
    dmask01 = np.stack([(j >= r * 128 + p).astype(np.float32)
                        for r in range(4)]).astype(BF)
    dmask01 = np.ascontiguousarray(dmask01)

    ON = (1.0, 0.0)
    OFF = (0.0, NEG)
    in_maps = []
    perms = []
    for core in range(8):
        b = core // 2
        z = core % 2
        blocks = [0, 3, 1, 2] if z == 0 else [1, 2, 0, 3]
        perms.append(blocks)
        cols = np.concatenate([np.arange(c * CH, (c + 1) * CH) for c in blocks])
        xT = np.ascontiguousarray(x[b].T[:, cols])
        sa = ON if blocks[2] < blocks[0] else OFF
        sbs = [ON if blocks[i] < blocks[1] else OFF for i in (0, 2, 3)]
        f = np.float32
        in_maps.append({
            "xT": xT,
            "w8_attn": w8a, "b_q": b_q, "b_v": b_v,
            "w_ap": w_ap_bf, "b_ap": b_ap,
            "ln1_g": ln1_g, "ln1_b": ln1_b, "ln2_g": ln2_g, "ln2_b": ln2_b,
            "w_fc": w_fc_bf, "b_fc": b_fc,
            "w_proj": w_pr_bf, "b_proj": b_pr,
            "dmask01": dmask01,
            "sA_scale": np.full((128, 1), sa[0], f),
            "sA_bias": np.full((128, 1), sa[1], f),
            "sB_scale": np.ascontiguousarray(
                np.tile(np.array([[s for s, _ in sbs]], f), (128, 1))),
            "sB_bias": np.ascontiguousarray(
                np.tile(np.array([[bb for _, bb in sbs]], f), (128, 1))),
        })
    return in_maps, perms


def _run(inputs, trace=False):
    from concourse.bass_utils import run_bass_kernel_spmd

    if "nc" not in _CACHE:
        _CACHE["nc"] = _build_program()
    nc = _CACHE["nc"]
    in_maps, perms = _host_prep(inputs)
    res = run_bass_kernel_spmd(nc, in_maps, list(range(8)), trace=trace)
    x = np.asarray(inputs["x"], np.float32)
    out = np.empty_like(x)
    for core in range(8):
        b = core // 2
        blocks = perms[core]
        oT = res.results[core]["outT"]
        cA, cB = blocks[0], blocks[1]
        out[b, cA * CH:(cA + 1) * CH, :] = oT[:, 0:CH].T
        out[b, cB * CH:(cB + 1) * CH, :] = oT[:, CH:2 * CH].T
    return out, res


def kernel(**inputs) -> np.ndarray:
    out, _ = _run(inputs, trace=False)
    return out
